# revision 11
# baseline (speedup 1.0000x reference)
import numpy as np

N = 100000
D = 64
NG = 64
NC = 8
NPC = N // NC          # 12500 real nodes per core
NB = 104               # blocks of 128 dst nodes per core
NPAD = NB * 128        # 13312 padded nodes per core
NCH = 4                # src chunks (2 cores each)
CHROWS = 2 * NPAD      # 26624 table rows per chunk (< int16 max)
import os as _os
GI = int(_os.environ.get("KERNEL_GI", "1024"))  # idxs per dma_gather instruction (HW max)
KSPLIT = int(_os.environ.get("KERNEL_KSPLIT", "4"))  # collective range splits
RG = [[0, 1, 2, 3, 4, 5, 6, 7]]


def _balance_core(v, cap=128, iters=4000, seed=0):
    """Assign nodes to NB blocks minimizing max (chunk, block) cell load.

    v: [n, NCH] per-node in-edge counts by source chunk (self-loops
    excluded). Returns (assign, loads).
    """
    n = len(v)
    tot = v.sum(1)
    order = np.argsort(-tot, kind="stable")
    loads = np.zeros((NB, NCH), np.float64)
    fill = np.zeros(NB, np.int64)
    assign = np.empty(n, np.int64)
    for i in order:
        w = v[i].astype(np.float64)
        sc = loads @ w + 1e18 * (fill >= cap)
        b = int(np.argmin(sc))
        assign[i] = b
        loads[b] += w
        fill[b] += 1
    loads = loads.astype(np.int64)
    rng = np.random.default_rng(seed)
    for _ in range(iters):
        over = loads.max(1)
        worst = over.max()
        bs = np.where(over == worst)[0]
        b = bs[rng.integers(len(bs))]
        d = np.argmax(loads[b])
        members = np.where(assign == b)[0]
        members = members[v[members, d] > 0]
        members = members[np.argsort(-v[members, d])][:30]
        moved = False
        for i in members:
            w = v[i]
            nm = (loads + w).max(1)
            nm[b] = 1 << 30
            nm = np.where(fill < cap, nm, 1 << 30)
            t = int(np.argmin(nm))
            if nm[t] < worst:
                assign[i] = t
                loads[b] -= w
                fill[b] -= 1
                loads[t] += w
                fill[t] += 1
                moved = True
                break
        if moved:
            continue
        done = False
        for i in members[:10]:
            w = v[i]
            for b2 in np.argsort(loads[:, d])[:10]:
                if b2 == b:
                    continue
                mem2 = np.where(assign == b2)[0]
                if not len(mem2):
                    continue
                m = mem2[np.argmin(v[mem2, d])]
                wm = v[m]
                nlb = loads[b] - w + wm
                nlb2 = loads[b2] + w - wm
                if nlb.max() < worst and nlb2.max() < worst:
                    assign[i] = b2
                    assign[m] = b
                    loads[b] = nlb
                    loads[b2] = nlb2
                    done = True
                    break
            if done:
                break
        if not done:
            break
    return assign, loads


def _preprocess(edge_index):
    src = edge_index[0].astype(np.int64)
    dst = edge_index[1].astype(np.int64)
    deg = np.bincount(dst, minlength=N) + 1  # +1 self loop
    dis = (1.0 / np.sqrt(deg.astype(np.float64))).astype(np.float32)

    core_of = np.arange(N) // NPC
    ch_of_src = core_of[src] // 2
    cnt4 = np.zeros((N, NCH), np.int64)
    np.add.at(cnt4, (dst, ch_of_src), 1)

    bb_of = np.empty(N, np.int64)
    p_of = np.empty(N, np.int64)
    worst = 0
    for c in range(NC):
        nodes = np.arange(c * NPC, (c + 1) * NPC)
        assign, loads = _balance_core(cnt4[nodes])
        worst = max(worst, int(loads.max()))
        bb_of[nodes] = assign
        # position within block
        for b in range(NB):
            mem = nodes[assign == b]
            p_of[mem] = np.arange(len(mem))
    T = ((max(worst, 256) + 127) // 128) * 128
    # bb-major rows within core: row = core*NPAD + bb*128 + p
    row_of = core_of * NPAD + bb_of * 128 + p_of

    NSLOT = NCH * NB * T
    gidx = np.zeros((NC, 128, NSLOT // 16), np.int16)
    dstl = np.full((NC, 128, NSLOT // 128), -1.0, np.float32)
    dst_core = core_of[dst]
    rng_s = np.random.default_rng(1)
    for c in range(NC):
        m = dst_core == c
        es, ed = src[m], dst[m]
        ch = core_of[es] // 2
        wrow = row_of[es] - ch * CHROWS
        gkey = ch * NB + bb_of[ed]
        if _os.environ.get("KERNEL_SORTROW", "1") == "1":
            o = np.lexsort((wrow, gkey))
        else:
            o = np.lexsort((rng_s.random(len(gkey)), gkey))
        gkey = gkey[o]
        wrow = wrow[o]
        pd = p_of[ed][o]
        cnt = np.bincount(gkey, minlength=NCH * NB)
        assert cnt.max() <= T, (cnt.max(), T)
        cum = np.zeros(NCH * NB + 1, np.int64)
        cum[1:] = np.cumsum(cnt)
        slot = gkey * T + (np.arange(len(gkey)) - cum[gkey])
        gi = np.zeros(NSLOT, np.int16)
        gi[slot] = wrow.astype(np.int16)
        gidx[c] = np.tile(gi.reshape(-1, 16).T, (8, 1))
        dl = np.full(NSLOT, -1.0, np.float32)
        dl[slot] = pd.astype(np.float32)
        dstl[c] = np.ascontiguousarray(dl.reshape(-1, 128).T)
    return dis, bb_of, p_of, T, gidx, dstl


def _build_program(T):
    import os
    from concourse import bacc, bass, mybir
    import concourse.tile as tile

    no_gather = os.environ.get("KERNEL_NO_GATHER", "") == "1"
    no_coll = os.environ.get("KERNEL_NO_COLL", "") == "1"
    no_agg = os.environ.get("KERNEL_NO_AGG", "") == "1"
    only_gather = os.environ.get("KERNEL_ONLY_GATHER", "") == "1"
    no_mm = os.environ.get("KERNEL_NO_MM", "") == "1"
    no_sel = os.environ.get("KERNEL_NO_SEL", "") == "1"
    no_epi = os.environ.get("KERNEL_NO_EPI", "") == "1"
    nqueues = int(os.environ.get("KERNEL_QUEUES", "4"))
    mtbufs = int(os.environ.get("KERNEL_MTBUFS", "3"))
    sp1 = os.environ.get("KERNEL_SP0", "") != "1"
    rfold = os.environ.get("KERNEL_RFOLD", "") == "1"
    psb = os.environ.get("KERNEL_PSB", "") == "1"
    sel1 = os.environ.get("KERNEL_SEL1", "") == "1"
    mb1 = os.environ.get("KERNEL_MB1", "") == "1"
    selbufs = int(os.environ.get("KERNEL_SELBUFS", "8"))
    reps = int(os.environ.get("KERNEL_REPS", "1"))
    K = KSPLIT
    assert NB % K == 0
    R = NB // K            # blocks per collective range

    f32 = mybir.dt.float32
    bf16 = mybir.dt.bfloat16
    i16 = mybir.dt.int16
    AF = mybir.ActivationFunctionType
    ALU = mybir.AluOpType
    NSLOT = NCH * NB * T
    M = T // 128          # matmul groups per bucket
    GPC = NB * T // GI    # gather instructions per chunk
    BPG = GI // T         # dst blocks per gather instruction

    nc = bacc.Bacc(None, target_bir_lowering=False, num_swdge_queues=nqueues)
    g0_h = nc.declare_dram_parameter("g0", [128, NB, 64], f32, False)
    g0bb_h = nc.declare_dram_parameter("g0bb", [NB * 128, 64], f32, False)
    wd_h = [
        nc.declare_dram_parameter(f"wd{i}", [128, 128], f32, False) for i in (1, 2)
    ]
    disc_h = nc.declare_dram_parameter("disc", [128, NB], f32, False)
    batc_h = nc.declare_dram_parameter("batc", [128, NB], f32, False)
    gidx_h = nc.declare_dram_parameter("gidx", [128, NSLOT // 16], i16, False)
    dstl_h = nc.declare_dram_parameter("dstl", [128, NSLOT // 128], bf16, False)
    w_h = [nc.declare_dram_parameter(f"w{i}", [D, D], f32, False) for i in range(3)]
    b_h = [nc.declare_dram_parameter(f"b{i}", [128, D], f32, False) for i in range(3)]
    iota_h = nc.declare_dram_parameter("iota", [128, BPG * M, 128], bf16, False)
    ident_h = nc.declare_dram_parameter("ident", [128, 128], f32, False)
    gid_h = nc.declare_dram_parameter("gid", [128, NG], f32, False)
    pooled_h = nc.declare_dram_parameter("pooled", [NG, D], f32, True)

    # bb-major shippable tables; layer 0's is copied from the g0bb param
    # (collectives may not read IO tensors)
    g_loc = [
        nc.dram_tensor(f"g_loc{L}", [NB * 128, 64], f32, kind="Internal")
        for L in range(3)
    ]
    g_full = [
        nc.dram_tensor(
            f"g_full{L}", [NC * NPAD, 64], f32, kind="Internal", addr_space="Shared"
        )
        for L in range(3)
    ]

    with tile.TileContext(nc) as tc:
        with tc.tile_pool(name="sb", bufs=1) as sb, tc.tile_pool(
            name="pp", bufs=1, space="PSUM"
        ) as pp:
            dis_sb = sb.tile([128, NB], f32)
            nc.sync.dma_start(out=dis_sb[:], in_=disc_h[:])
            bat_sb = sb.tile([128, NB], f32)
            nc.sync.dma_start(out=bat_sb[:], in_=batc_h[:])
            gidx_sb = sb.tile([128, NSLOT // 16], i16)
            nc.sync.dma_start(out=gidx_sb[:], in_=gidx_h[:])
            dstl_sb = sb.tile([128, NSLOT // 128], bf16)
            nc.sync.dma_start(out=dstl_sb[:], in_=dstl_h[:])
            w_sb, b_sb = [], []
            for i in range(3):
                wt = sb.tile([D, D], f32, name=f"w_sb{i}")
                nc.sync.dma_start(out=wt[:], in_=w_h[i][:])
                w_sb.append(wt)
                bt = sb.tile([128, D], f32, name=f"b_sb{i}")
                nc.sync.dma_start(out=bt[:], in_=b_h[i][:])
                b_sb.append(bt)
            wd_sb = [None]
            for i in (1, 2):
                wdf = sb.tile([128, 128], f32, name=f"wdf{i}")
                nc.sync.dma_start(out=wdf[:], in_=wd_h[i - 1][:])
                wdt = sb.tile([128, 128], bf16, name=f"wd_sb{i}")
                nc.scalar.activation(out=wdt[:], in_=wdf[:], func=AF.Copy)
                wd_sb.append(wdt)
            iota_sb = sb.tile([128, BPG * M, 128], bf16)
            nc.sync.dma_start(out=iota_sb[:], in_=iota_h[:])
            ident_sb = sb.tile([128, 128], f32)
            nc.sync.dma_start(out=ident_sb[:], in_=ident_h[:])
            identb_sb = sb.tile([128, 128], bf16)
            nc.scalar.activation(out=identb_sb[:], in_=ident_sb[:], func=AF.Copy)
            gid_sb = sb.tile([128, NG], f32)
            nc.sync.dma_start(out=gid_sb[:], in_=gid_h[:])

            GA = sb.tile([128, NB, 64], f32, name="GA")
            GB = sb.tile([128, NB, 64], f32, name="GB")
            nc.sync.dma_start(out=GA[:], in_=g0_h[:])
            nc.sync.dma_start(out=g_loc[0][:], in_=g0bb_h[:])
            gi_reg = nc.gpsimd.to_reg(GI)

            for L in [Li for _ in range(reps) for Li in range(3)]:
                G_sb = [GA, GB, GA][L]
                G_next = [GB, GA, None][L]
                # range-split AllGathers into strided slices of g_full[L].
                # Range k's input is ready as soon as the previous layer's
                # aggregation finished blocks [k*R, (k+1)*R), so early
                # ranges overlap the previous layer's tail compute.
                if no_coll:
                    nc.sync.dma_start(
                        out=g_full[L][: R * 128, :],
                        in_=g_loc[L][: R * 128, :],
                    )
                else:
                    nc.gpsimd.collective_compute(
                        "AllGather",
                        ALU.bypass,
                        replica_groups=RG,
                        ins=[g_loc[L][:]],
                        outs=[g_full[L][:]],
                    )
                # After the table is shipped, overwrite G_sb in place with
                # pre = dis*G + b (self-loop + bias term), two big DVE ops.
                nc.vector.tensor_tensor(
                    out=G_sb[:],
                    in0=G_sb[:],
                    in1=dis_sb[:, :].to_broadcast([128, NB, 64]),
                    op=ALU.mult,
                )
                b_ap = b_sb[L][:]
                b_bcast = bass.AP(
                    b_ap.tensor,
                    b_ap.offset,
                    [[b_ap.ap[0][0], 128], [0, NB], [1, 64]],
                )
                nc.vector.tensor_tensor(
                    out=G_sb[:], in0=G_sb[:], in1=b_bcast, op=ALU.add
                )
                if L < 2:
                    pool_ps = None
                    oh_all = None
                else:
                    pool_ps = pp.tile([NG, D], f32)
                    # one-hot graph-membership for all blocks in one op
                    oh_all = sb.tile([128, NB, NG], bf16, name="oh_all")
                    g_ap = gid_sb[:]
                    gid_bcast = bass.AP(
                        g_ap.tensor,
                        g_ap.offset,
                        [[g_ap.ap[0][0], 128], [0, NB], [1, NG]],
                    )
                    nc.vector.tensor_tensor(
                        out=oh_all[:],
                        in0=bat_sb[:, :].to_broadcast([128, NB, NG]),
                        in1=gid_bcast,
                        op=ALU.is_equal,
                    )

                MI = GI // 128
                shipped = 0
                for gp in range(GPC):  # gather-instruction groups
                    mtall = sb.tile([128, NCH * MI, 64], f32, bufs=mtbufs, name="mt")
                    if no_gather or no_agg:
                        nc.vector.memset(mtall[:], 0.0)
                    else:
                        for ch in range(NCH):
                            win = g_full[L][ch * CHROWS : (ch + 1) * CHROWS, :]
                            s0 = ch * NB * T + gp * GI
                            nc.gpsimd.dma_gather(
                                out_ap=mtall[:, ch * MI : (ch + 1) * MI, :],
                                in_ap=win,
                                idxs_ap=gidx_sb[:, s0 // 16 : (s0 + GI) // 16],
                                num_idxs=GI,
                                num_idxs_reg=gi_reg,
                                elem_size=64,
                                single_packet=sp1,
                                queue_num=ch % nqueues,
                            )
                    mball = sb.tile([128, NCH * MI, 64], bf16, bufs=mtbufs, name="mb")
                    if not only_gather:
                        # per-chunk copies so each chunk's matmuls unblock as
                        # soon as that chunk's gather lands
                        if mb1:
                            nc.scalar.activation(
                                out=mball[:], in_=mtall[:], func=AF.Copy
                            )
                        else:
                            for ch in range(NCH):
                                nc.scalar.activation(
                                    out=mball[:, ch * MI : (ch + 1) * MI, :],
                                    in_=mtall[:, ch * MI : (ch + 1) * MI, :],
                                    func=AF.Copy,
                                )

                    sel3s = []
                    sel3a = None
                    if not (no_agg or only_gather or no_sel):
                        if sel1:
                            sel3a = sb.tile(
                                [128, NCH, BPG * M, 128], bf16,
                                bufs=max(selbufs // 2, 2), name="sel3a",
                            )
                            d_ap = dstl_sb[:]
                            dcol0 = gp * BPG * M
                            in0 = bass.AP(
                                d_ap.tensor,
                                d_ap.offset + dcol0,
                                [[d_ap.ap[0][0], 128], [NB * M, NCH],
                                 [1, BPG * M], [0, 128]],
                            )
                            i_ap = iota_sb[:]
                            in1 = bass.AP(
                                i_ap.tensor,
                                i_ap.offset,
                                [[i_ap.ap[0][0], 128], [0, NCH],
                                 [128, BPG * M], [1, 128]],
                            )
                            nc.vector.tensor_tensor(
                                out=sel3a[:], in0=in0, in1=in1, op=ALU.is_equal
                            )
                        else:
                            for ch in range(NCH):
                                sel3 = sb.tile([128, BPG * M, 128], bf16,
                                               bufs=selbufs, name="sel3")
                                dcol = (ch * NB + gp * BPG) * M
                                nc.vector.tensor_tensor(
                                    out=sel3[:],
                                    in0=dstl_sb[:, dcol : dcol + BPG * M]
                                    .to_broadcast([128, BPG * M, 128]),
                                    in1=iota_sb[:],
                                    op=ALU.is_equal,
                                )
                                sel3s.append(sel3)

                    bb0 = gp * BPG
                    accs = []
                    for bi in range(BPG):
                        acc = pp.tile([128, D], f32, bufs=2 if psb else 3, name="acc")
                        if no_agg or only_gather or no_mm or no_sel:
                            nc.vector.memset(acc[:], 0.0)
                        else:
                            for ch in range(NCH):
                                for m in range(M):
                                    nc.tensor.matmul(
                                        out=acc[:],
                                        lhsT=(
                                            sel3a[:, ch, bi * M + m, :]
                                            if sel1
                                            else sel3s[ch][:, bi * M + m, :]
                                        ),
                                        rhs=mball[:, ch * MI + bi * M + m, :],
                                        start=(ch == 0 and m == 0),
                                        stop=(ch == NCH - 1 and m == M - 1),
                                    )
                        accs.append(acc)
                    # h = relu(dis*acc + (dis*G + b)), both blocks at once
                    t1 = sb.tile([128, BPG, D], f32, bufs=3, name="t1")
                    for bi in range(BPG):
                        nc.vector.tensor_scalar(
                            out=t1[:, bi, :],
                            in0=accs[bi][:],
                            scalar1=dis_sb[:, bb0 + bi : bb0 + bi + 1],
                            scalar2=None,
                            op0=ALU.mult,
                        )
                    nc.vector.tensor_add(
                        out=t1[:], in0=t1[:], in1=G_sb[:, bb0 : bb0 + BPG, :]
                    )
                    if L == 2 or not rfold:
                        h_sb = sb.tile([128, BPG, D], bf16, bufs=3, name="h")
                        nc.scalar.activation(out=h_sb[:], in_=t1[:], func=AF.Relu)
                    if no_epi:
                        if L == 2 and gp == GPC - 1:
                            nc.tensor.matmul(
                                out=pool_ps[:], lhsT=oh_all[:, bb0, :],
                                rhs=h_sb[:, 0, :], start=True, stop=True,
                            )
                    elif L < 2:
                        # transpose both blocks: [(bi,f), pos]; with rfold the
                        # relu commutes past the transpose into the hTb copy
                        if rfold:
                            tps = pp.tile([128, 128], f32, bufs=3 if psb else 2)
                            nc.tensor.transpose(
                                out=tps[:], in_=t1[:, :, :], identity=ident_sb[:]
                            )
                            hTb = sb.tile([128, 128], bf16, bufs=3, name="hTb")
                            nc.scalar.activation(
                                out=hTb[:], in_=tps[:], func=AF.Relu
                            )
                        else:
                            tps = pp.tile([128, 128], bf16, bufs=3 if psb else 2)
                            nc.tensor.transpose(
                                out=tps[:], in_=h_sb[:, :, :], identity=identb_sb[:]
                            )
                            hTb = sb.tile([128, 128], bf16, bufs=3, name="hTb")
                            nc.scalar.activation(
                                out=hTb[:], in_=tps[:], func=AF.Copy
                            )
                        # block-diagonal W applies h@W to both blocks in one mm
                        gps2 = pp.tile([128, BPG, D], f32, bufs=2)
                        nc.tensor.matmul(
                            out=gps2[:, :, :],
                            lhsT=hTb[:],
                            rhs=wd_sb[L + 1][:],
                            start=True,
                            stop=True,
                        )
                        nc.vector.tensor_tensor(
                            out=G_next[:, bb0 : bb0 + BPG, :],
                            in0=gps2[:, :, :],
                            in1=dis_sb[:, bb0 : bb0 + BPG].to_broadcast(
                                [128, BPG, 64]
                            ),
                            op=ALU.mult,
                        )
                    else:
                        for bi in range(BPG):
                            bb = bb0 + bi
                            nc.tensor.matmul(
                                out=pool_ps[:],
                                lhsT=oh_all[:, bb, :],
                                rhs=h_sb[:, bi, :],
                                start=(bb == 0),
                                stop=(bb == NB - 1),
                            )
                    # ship any completed range of next layer's table
                    if L < 2:
                        done = (gp + 1) * BPG
                        while shipped < K and (shipped + 1) * R <= done:
                            k = shipped
                            dst_t = g_loc[L + 1][:]
                            dst_ap = bass.AP(
                                dst_t.tensor,
                                k * R * 128 * 64,
                                [[64, 128], [128 * 64, R], [1, 64]],
                            )
                            nc.sync.dma_start(
                                out=dst_ap,
                                in_=G_next[:, k * R : (k + 1) * R, :],
                            )
                            shipped += 1
                if L == 2:
                    pool_sb = sb.tile([NG, D], f32)
                    nc.scalar.activation(
                        out=pool_sb[:], in_=pool_ps[:], func=AF.Copy
                    )
                    nc.sync.dma_start(out=pooled_h[:], in_=pool_sb[:])
    if not nc.is_finalized():
        nc.finalize()
    return nc


LAST_RESULTS = None
LAST_PREP = None


def prepare(inputs):
    x = np.asarray(inputs["x"], np.float32)
    edge_index = np.asarray(inputs["edge_index"])
    batch = np.asarray(inputs["batch"])
    W = [np.asarray(inputs[k], np.float32) for k in ("W1", "W2", "W3")]
    b = [np.asarray(inputs[k], np.float32) for k in ("b1", "b2", "b3")]
    lin_w = np.asarray(inputs["lin_w"], np.float32)
    lin_b = np.asarray(inputs["lin_b"], np.float32)

    dis, bb_of, p_of, T, gidx, dstl = _preprocess(edge_index)

    g0_all = (x @ W[0]) * dis[:, None]  # layer-0 table, exact fp32
    g0 = np.zeros((NC, 128, NB, 64), np.float32)
    g0bb = np.zeros((NC, NB * 128, 64), np.float32)
    disc = np.zeros((NC, 128, NB), np.float32)
    batc = np.full((NC, 128, NB), -1.0, np.float32)
    for c in range(NC):
        nodes = np.arange(c * NPC, (c + 1) * NPC)
        g0[c][p_of[nodes], bb_of[nodes]] = g0_all[nodes]
        g0bb[c][bb_of[nodes] * 128 + p_of[nodes]] = g0_all[nodes]
        disc[c][p_of[nodes], bb_of[nodes]] = dis[nodes]
        batc[c][p_of[nodes], bb_of[nodes]] = batch[nodes].astype(np.float32)

    M = T // 128
    BPGh = max(GI // T, 1)
    import ml_dtypes
    iota = np.ascontiguousarray(
        np.broadcast_to(np.arange(128, dtype=np.float32), (128, BPGh * M, 128))
        .copy()
        .astype(ml_dtypes.bfloat16)
    )
    ident = np.eye(128, dtype=np.float32)
    gid = np.ascontiguousarray(np.tile(np.arange(NG, dtype=np.float32), (128, 1)))
    b_repl = [np.ascontiguousarray(np.tile(bi.reshape(1, D), (128, 1))) for bi in b]
    z = np.zeros((D, D), np.float32)
    wd = [
        np.ascontiguousarray(np.block([[Wi, z], [z, Wi]])) for Wi in (W[1], W[2])
    ]

    nc = _build_program(T)
    in_maps = []
    for c in range(NC):
        in_maps.append(
            {
                "g0": np.ascontiguousarray(g0[c]),
                "g0bb": np.ascontiguousarray(g0bb[c]),
                "disc": np.ascontiguousarray(disc[c]),
                "batc": np.ascontiguousarray(batc[c]),
                "gidx": np.ascontiguousarray(gidx[c]),
                "dstl": np.ascontiguousarray(dstl[c]).astype(ml_dtypes.bfloat16),
                "w0": W[0],
                "w1": W[1],
                "w2": W[2],
                "wd1": wd[0],
                "wd2": wd[1],
                "b0": b_repl[0],
                "b1": b_repl[1],
                "b2": b_repl[2],
                "iota": iota,
                "ident": ident,
                "gid": gid,
            }
        )

    def post(results):
        pooled = np.zeros((NG, D), np.float64)
        for r in results:
            pooled += r["pooled"].astype(np.float64)
        out = pooled.astype(np.float32) @ lin_w + lin_b
        return out.astype(np.float32)

    return nc, in_maps, post


def kernel(**inputs):
    import os
    from concourse.bass_utils import run_bass_kernel_spmd

    nc, in_maps, post = prepare(inputs)
    global LAST_PREP
    LAST_PREP = (nc, in_maps, post)
    trace = os.environ.get("KERNEL_TRACE", "") == "1"
    res = run_bass_kernel_spmd(nc, in_maps, list(range(NC)), trace=trace)
    global LAST_RESULTS
    LAST_RESULTS = res
    return post(res.results)


# revision 12
# speedup vs baseline: 1.0229x; 1.0229x over previous
import numpy as np

N = 100000
D = 64
NG = 64
NC = 8
NPC = N // NC          # 12500 real nodes per core
NB = 104               # blocks of 128 dst nodes per core
NPAD = NB * 128        # 13312 padded nodes per core
NCH = 4                # src chunks (2 cores each)
CHROWS = 2 * NPAD      # 26624 table rows per chunk (< int16 max)
import os as _os
GI = int(_os.environ.get("KERNEL_GI", "1024"))  # idxs per dma_gather instruction (HW max)
KSPLIT = int(_os.environ.get("KERNEL_KSPLIT", "4"))  # collective range splits
RG = [[0, 1, 2, 3, 4, 5, 6, 7]]


def _balance_core(v, cap=128, iters=4000, seed=0):
    """Assign nodes to NB blocks minimizing max (chunk, block) cell load.

    v: [n, NCH] per-node in-edge counts by source chunk (self-loops
    excluded). Returns (assign, loads).
    """
    n = len(v)
    tot = v.sum(1)
    order = np.argsort(-tot, kind="stable")
    loads = np.zeros((NB, NCH), np.float64)
    fill = np.zeros(NB, np.int64)
    assign = np.empty(n, np.int64)
    for i in order:
        w = v[i].astype(np.float64)
        sc = loads @ w + 1e18 * (fill >= cap)
        b = int(np.argmin(sc))
        assign[i] = b
        loads[b] += w
        fill[b] += 1
    loads = loads.astype(np.int64)
    rng = np.random.default_rng(seed)
    for _ in range(iters):
        over = loads.max(1)
        worst = over.max()
        bs = np.where(over == worst)[0]
        b = bs[rng.integers(len(bs))]
        d = np.argmax(loads[b])
        members = np.where(assign == b)[0]
        members = members[v[members, d] > 0]
        members = members[np.argsort(-v[members, d])][:30]
        moved = False
        for i in members:
            w = v[i]
            nm = (loads + w).max(1)
            nm[b] = 1 << 30
            nm = np.where(fill < cap, nm, 1 << 30)
            t = int(np.argmin(nm))
            if nm[t] < worst:
                assign[i] = t
                loads[b] -= w
                fill[b] -= 1
                loads[t] += w
                fill[t] += 1
                moved = True
                break
        if moved:
            continue
        done = False
        for i in members[:10]:
            w = v[i]
            for b2 in np.argsort(loads[:, d])[:10]:
                if b2 == b:
                    continue
                mem2 = np.where(assign == b2)[0]
                if not len(mem2):
                    continue
                m = mem2[np.argmin(v[mem2, d])]
                wm = v[m]
                nlb = loads[b] - w + wm
                nlb2 = loads[b2] + w - wm
                if nlb.max() < worst and nlb2.max() < worst:
                    assign[i] = b2
                    assign[m] = b
                    loads[b] = nlb
                    loads[b2] = nlb2
                    done = True
                    break
            if done:
                break
        if not done:
            break
    return assign, loads


def _preprocess(edge_index):
    src = edge_index[0].astype(np.int64)
    dst = edge_index[1].astype(np.int64)
    deg = np.bincount(dst, minlength=N) + 1  # +1 self loop
    dis = (1.0 / np.sqrt(deg.astype(np.float64))).astype(np.float32)

    core_of = np.arange(N) // NPC
    ch_of_src = core_of[src] // 2
    cnt4 = np.zeros((N, NCH), np.int64)
    np.add.at(cnt4, (dst, ch_of_src), 1)

    bb_of = np.empty(N, np.int64)
    p_of = np.empty(N, np.int64)
    worst = 0
    for c in range(NC):
        nodes = np.arange(c * NPC, (c + 1) * NPC)
        assign, loads = _balance_core(cnt4[nodes])
        worst = max(worst, int(loads.max()))
        bb_of[nodes] = assign
        # position within block
        for b in range(NB):
            mem = nodes[assign == b]
            p_of[mem] = np.arange(len(mem))
    T = ((max(worst, 256) + 127) // 128) * 128
    # bb-major rows within core: row = core*NPAD + bb*128 + p
    row_of = core_of * NPAD + bb_of * 128 + p_of

    NSLOT = NCH * NB * T
    gidx = np.zeros((NC, 128, NSLOT // 16), np.int16)
    dstl = np.full((NC, 128, NSLOT // 128), -1.0, np.float32)
    dst_core = core_of[dst]
    rng_s = np.random.default_rng(1)
    for c in range(NC):
        m = dst_core == c
        es, ed = src[m], dst[m]
        ch = core_of[es] // 2
        wrow = row_of[es] - ch * CHROWS
        gkey = ch * NB + bb_of[ed]
        if _os.environ.get("KERNEL_SORTROW", "1") == "1":
            o = np.lexsort((wrow, gkey))
        else:
            o = np.lexsort((rng_s.random(len(gkey)), gkey))
        gkey = gkey[o]
        wrow = wrow[o]
        pd = p_of[ed][o]
        cnt = np.bincount(gkey, minlength=NCH * NB)
        assert cnt.max() <= T, (cnt.max(), T)
        cum = np.zeros(NCH * NB + 1, np.int64)
        cum[1:] = np.cumsum(cnt)
        slot = gkey * T + (np.arange(len(gkey)) - cum[gkey])
        gi = np.zeros(NSLOT, np.int16)
        gi[slot] = wrow.astype(np.int16)
        gidx[c] = np.tile(gi.reshape(-1, 16).T, (8, 1))
        dl = np.full(NSLOT, -1.0, np.float32)
        dl[slot] = pd.astype(np.float32)
        dstl[c] = np.ascontiguousarray(dl.reshape(-1, 128).T)
    return dis, bb_of, p_of, T, gidx, dstl


def _build_program(T):
    import os
    from concourse import bacc, bass, mybir
    import concourse.tile as tile

    no_gather = os.environ.get("KERNEL_NO_GATHER", "") == "1"
    no_coll = os.environ.get("KERNEL_NO_COLL", "") == "1"
    no_agg = os.environ.get("KERNEL_NO_AGG", "") == "1"
    only_gather = os.environ.get("KERNEL_ONLY_GATHER", "") == "1"
    no_mm = os.environ.get("KERNEL_NO_MM", "") == "1"
    no_sel = os.environ.get("KERNEL_NO_SEL", "") == "1"
    no_epi = os.environ.get("KERNEL_NO_EPI", "") == "1"
    nqueues = int(os.environ.get("KERNEL_QUEUES", "4"))
    mtbufs = int(os.environ.get("KERNEL_MTBUFS", "3"))
    sp1 = os.environ.get("KERNEL_SP0", "") != "1"
    rfold = os.environ.get("KERNEL_RFOLD", "") == "1"
    psb = os.environ.get("KERNEL_PSB", "") == "1"
    sel1 = os.environ.get("KERNEL_SEL1", "") == "1"
    mb1 = os.environ.get("KERNEL_MB1", "") == "1"
    epibufs = int(os.environ.get("KERNEL_EPIBUFS", "3"))
    selbufs = int(os.environ.get("KERNEL_SELBUFS", "8"))
    reps = int(os.environ.get("KERNEL_REPS", "1"))
    K = KSPLIT
    assert NB % K == 0
    R = NB // K            # blocks per collective range

    f32 = mybir.dt.float32
    bf16 = mybir.dt.bfloat16
    i16 = mybir.dt.int16
    AF = mybir.ActivationFunctionType
    ALU = mybir.AluOpType
    NSLOT = NCH * NB * T
    M = T // 128          # matmul groups per bucket
    GPC = NB * T // GI    # gather instructions per chunk
    BPG = GI // T         # dst blocks per gather instruction

    nc = bacc.Bacc(None, target_bir_lowering=False, num_swdge_queues=nqueues)
    g0_h = nc.declare_dram_parameter("g0", [128, NB, 64], f32, False)
    g0bb_h = nc.declare_dram_parameter("g0bb", [NB * 128, 64], f32, False)
    wd_h = [
        nc.declare_dram_parameter(f"wd{i}", [128, 128], f32, False) for i in (1, 2)
    ]
    disc_h = nc.declare_dram_parameter("disc", [128, NB], f32, False)
    batc_h = nc.declare_dram_parameter("batc", [128, NB], f32, False)
    gidx_h = nc.declare_dram_parameter("gidx", [128, NSLOT // 16], i16, False)
    dstl_h = nc.declare_dram_parameter("dstl", [128, NSLOT // 128], bf16, False)
    w_h = [nc.declare_dram_parameter(f"w{i}", [D, D], f32, False) for i in range(3)]
    b_h = [nc.declare_dram_parameter(f"b{i}", [128, D], f32, False) for i in range(3)]
    iota_h = nc.declare_dram_parameter("iota", [128, BPG * M, 128], bf16, False)
    ident_h = nc.declare_dram_parameter("ident", [128, 128], f32, False)
    gid_h = nc.declare_dram_parameter("gid", [128, NG], f32, False)
    pooled_h = nc.declare_dram_parameter("pooled", [NG, D], f32, True)

    # bb-major shippable tables; layer 0's is copied from the g0bb param
    # (collectives may not read IO tensors)
    g_loc = [
        nc.dram_tensor(f"g_loc{L}", [NB * 128, 64], f32, kind="Internal")
        for L in range(3)
    ]
    g_full = [
        nc.dram_tensor(
            f"g_full{L}", [NC * NPAD, 64], f32, kind="Internal", addr_space="Shared"
        )
        for L in range(3)
    ]

    with tile.TileContext(nc) as tc:
        with tc.tile_pool(name="sb", bufs=1) as sb, tc.tile_pool(
            name="pp", bufs=1, space="PSUM"
        ) as pp:
            dis_sb = sb.tile([128, NB], f32)
            nc.sync.dma_start(out=dis_sb[:], in_=disc_h[:])
            bat_sb = sb.tile([128, NB], f32)
            nc.sync.dma_start(out=bat_sb[:], in_=batc_h[:])
            gidx_sb = sb.tile([128, NSLOT // 16], i16)
            nc.sync.dma_start(out=gidx_sb[:], in_=gidx_h[:])
            dstl_sb = sb.tile([128, NSLOT // 128], bf16)
            nc.sync.dma_start(out=dstl_sb[:], in_=dstl_h[:])
            w_sb, b_sb = [], []
            for i in range(3):
                wt = sb.tile([D, D], f32, name=f"w_sb{i}")
                nc.sync.dma_start(out=wt[:], in_=w_h[i][:])
                w_sb.append(wt)
                bt = sb.tile([128, D], f32, name=f"b_sb{i}")
                nc.sync.dma_start(out=bt[:], in_=b_h[i][:])
                b_sb.append(bt)
            wd_sb = [None]
            for i in (1, 2):
                wdf = sb.tile([128, 128], f32, name=f"wdf{i}")
                nc.sync.dma_start(out=wdf[:], in_=wd_h[i - 1][:])
                wdt = sb.tile([128, 128], bf16, name=f"wd_sb{i}")
                nc.scalar.activation(out=wdt[:], in_=wdf[:], func=AF.Copy)
                wd_sb.append(wdt)
            iota_sb = sb.tile([128, BPG * M, 128], bf16)
            nc.sync.dma_start(out=iota_sb[:], in_=iota_h[:])
            ident_sb = sb.tile([128, 128], f32)
            nc.sync.dma_start(out=ident_sb[:], in_=ident_h[:])
            identb_sb = sb.tile([128, 128], bf16)
            nc.scalar.activation(out=identb_sb[:], in_=ident_sb[:], func=AF.Copy)
            gid_sb = sb.tile([128, NG], f32)
            nc.sync.dma_start(out=gid_sb[:], in_=gid_h[:])

            GA = sb.tile([128, NB, 64], f32, name="GA")
            GB = sb.tile([128, NB, 64], f32, name="GB")
            nc.sync.dma_start(out=GA[:], in_=g0_h[:])
            nc.sync.dma_start(out=g_loc[0][:], in_=g0bb_h[:])
            gi_reg = nc.gpsimd.to_reg(GI)

            for L in [Li for _ in range(reps) for Li in range(3)]:
                G_sb = [GA, GB, GA][L]
                G_next = [GB, GA, None][L]
                # range-split AllGathers into strided slices of g_full[L].
                # Range k's input is ready as soon as the previous layer's
                # aggregation finished blocks [k*R, (k+1)*R), so early
                # ranges overlap the previous layer's tail compute.
                if no_coll:
                    nc.sync.dma_start(
                        out=g_full[L][: R * 128, :],
                        in_=g_loc[L][: R * 128, :],
                    )
                else:
                    nc.gpsimd.collective_compute(
                        "AllGather",
                        ALU.bypass,
                        replica_groups=RG,
                        ins=[g_loc[L][:]],
                        outs=[g_full[L][:]],
                    )
                # After the table is shipped, overwrite G_sb in place with
                # pre = dis*G + b (self-loop + bias term), two big DVE ops.
                nc.vector.tensor_tensor(
                    out=G_sb[:],
                    in0=G_sb[:],
                    in1=dis_sb[:, :].to_broadcast([128, NB, 64]),
                    op=ALU.mult,
                )
                b_ap = b_sb[L][:]
                b_bcast = bass.AP(
                    b_ap.tensor,
                    b_ap.offset,
                    [[b_ap.ap[0][0], 128], [0, NB], [1, 64]],
                )
                nc.vector.tensor_tensor(
                    out=G_sb[:], in0=G_sb[:], in1=b_bcast, op=ALU.add
                )
                if L < 2:
                    pool_ps = None
                    oh_all = None
                else:
                    pool_ps = pp.tile([NG, D], f32)
                    # one-hot graph-membership for all blocks in one op
                    oh_all = sb.tile([128, NB, NG], bf16, name="oh_all")
                    g_ap = gid_sb[:]
                    gid_bcast = bass.AP(
                        g_ap.tensor,
                        g_ap.offset,
                        [[g_ap.ap[0][0], 128], [0, NB], [1, NG]],
                    )
                    nc.vector.tensor_tensor(
                        out=oh_all[:],
                        in0=bat_sb[:, :].to_broadcast([128, NB, NG]),
                        in1=gid_bcast,
                        op=ALU.is_equal,
                    )

                MI = GI // 128
                shipped = 0
                for gp in range(GPC):  # gather-instruction groups
                    mtall = sb.tile([128, NCH * MI, 64], f32, bufs=mtbufs, name="mt")
                    if no_gather or no_agg:
                        nc.vector.memset(mtall[:], 0.0)
                    else:
                        for ch in range(NCH):
                            win = g_full[L][ch * CHROWS : (ch + 1) * CHROWS, :]
                            s0 = ch * NB * T + gp * GI
                            nc.gpsimd.dma_gather(
                                out_ap=mtall[:, ch * MI : (ch + 1) * MI, :],
                                in_ap=win,
                                idxs_ap=gidx_sb[:, s0 // 16 : (s0 + GI) // 16],
                                num_idxs=GI,
                                num_idxs_reg=gi_reg,
                                elem_size=64,
                                single_packet=sp1,
                                queue_num=ch % nqueues,
                            )
                    mball = sb.tile([128, NCH * MI, 64], bf16, bufs=mtbufs, name="mb")
                    if not only_gather:
                        # per-chunk copies so each chunk's matmuls unblock as
                        # soon as that chunk's gather lands
                        if mb1:
                            nc.scalar.activation(
                                out=mball[:], in_=mtall[:], func=AF.Copy
                            )
                        else:
                            for ch in range(NCH):
                                nc.scalar.activation(
                                    out=mball[:, ch * MI : (ch + 1) * MI, :],
                                    in_=mtall[:, ch * MI : (ch + 1) * MI, :],
                                    func=AF.Copy,
                                )

                    sel3s = []
                    sel3a = None
                    if not (no_agg or only_gather or no_sel):
                        if sel1:
                            sel3a = sb.tile(
                                [128, NCH, BPG * M, 128], bf16,
                                bufs=max(selbufs // 2, 2), name="sel3a",
                            )
                            d_ap = dstl_sb[:]
                            dcol0 = gp * BPG * M
                            in0 = bass.AP(
                                d_ap.tensor,
                                d_ap.offset + dcol0,
                                [[d_ap.ap[0][0], 128], [NB * M, NCH],
                                 [1, BPG * M], [0, 128]],
                            )
                            i_ap = iota_sb[:]
                            in1 = bass.AP(
                                i_ap.tensor,
                                i_ap.offset,
                                [[i_ap.ap[0][0], 128], [0, NCH],
                                 [128, BPG * M], [1, 128]],
                            )
                            nc.vector.tensor_tensor(
                                out=sel3a[:], in0=in0, in1=in1, op=ALU.is_equal
                            )
                        else:
                            for ch in range(NCH):
                                sel3 = sb.tile([128, BPG * M, 128], bf16,
                                               bufs=selbufs, name="sel3")
                                dcol = (ch * NB + gp * BPG) * M
                                nc.vector.tensor_tensor(
                                    out=sel3[:],
                                    in0=dstl_sb[:, dcol : dcol + BPG * M]
                                    .to_broadcast([128, BPG * M, 128]),
                                    in1=iota_sb[:],
                                    op=ALU.is_equal,
                                )
                                sel3s.append(sel3)

                    bb0 = gp * BPG
                    accs = []
                    for bi in range(BPG):
                        acc = pp.tile([128, D], f32, bufs=2 if psb else 3, name="acc")
                        if no_agg or only_gather or no_mm or no_sel:
                            nc.vector.memset(acc[:], 0.0)
                        else:
                            for ch in range(NCH):
                                for m in range(M):
                                    nc.tensor.matmul(
                                        out=acc[:],
                                        lhsT=(
                                            sel3a[:, ch, bi * M + m, :]
                                            if sel1
                                            else sel3s[ch][:, bi * M + m, :]
                                        ),
                                        rhs=mball[:, ch * MI + bi * M + m, :],
                                        start=(ch == 0 and m == 0),
                                        stop=(ch == NCH - 1 and m == M - 1),
                                    )
                        accs.append(acc)
                    # h = relu(dis*acc + (dis*G + b)), both blocks at once
                    t1 = sb.tile([128, BPG, D], f32, bufs=epibufs, name="t1")
                    for bi in range(BPG):
                        nc.vector.tensor_scalar(
                            out=t1[:, bi, :],
                            in0=accs[bi][:],
                            scalar1=dis_sb[:, bb0 + bi : bb0 + bi + 1],
                            scalar2=None,
                            op0=ALU.mult,
                        )
                    nc.vector.tensor_add(
                        out=t1[:], in0=t1[:], in1=G_sb[:, bb0 : bb0 + BPG, :]
                    )
                    if L == 2 or not rfold:
                        h_sb = sb.tile([128, BPG, D], bf16, bufs=epibufs, name="h")
                        nc.scalar.activation(out=h_sb[:], in_=t1[:], func=AF.Relu)
                    if no_epi:
                        if L == 2 and gp == GPC - 1:
                            nc.tensor.matmul(
                                out=pool_ps[:], lhsT=oh_all[:, bb0, :],
                                rhs=h_sb[:, 0, :], start=True, stop=True,
                            )
                    elif L < 2:
                        # transpose both blocks: [(bi,f), pos]; with rfold the
                        # relu commutes past the transpose into the hTb copy
                        if rfold:
                            tps = pp.tile([128, 128], f32, bufs=3 if psb else 2)
                            nc.tensor.transpose(
                                out=tps[:], in_=t1[:, :, :], identity=ident_sb[:]
                            )
                            hTb = sb.tile([128, 128], bf16, bufs=epibufs, name="hTb")
                            nc.scalar.activation(
                                out=hTb[:], in_=tps[:], func=AF.Relu
                            )
                        else:
                            tps = pp.tile([128, 128], bf16, bufs=3 if psb else 2)
                            nc.tensor.transpose(
                                out=tps[:], in_=h_sb[:, :, :], identity=identb_sb[:]
                            )
                            hTb = sb.tile([128, 128], bf16, bufs=epibufs, name="hTb")
                            nc.scalar.activation(
                                out=hTb[:], in_=tps[:], func=AF.Copy
                            )
                        # block-diagonal W applies h@W to both blocks in one mm
                        gps2 = pp.tile([128, BPG, D], f32, bufs=2)
                        nc.tensor.matmul(
                            out=gps2[:, :, :],
                            lhsT=hTb[:],
                            rhs=wd_sb[L + 1][:],
                            start=True,
                            stop=True,
                        )
                        nc.vector.tensor_tensor(
                            out=G_next[:, bb0 : bb0 + BPG, :],
                            in0=gps2[:, :, :],
                            in1=dis_sb[:, bb0 : bb0 + BPG].to_broadcast(
                                [128, BPG, 64]
                            ),
                            op=ALU.mult,
                        )
                    else:
                        for bi in range(BPG):
                            bb = bb0 + bi
                            nc.tensor.matmul(
                                out=pool_ps[:],
                                lhsT=oh_all[:, bb, :],
                                rhs=h_sb[:, bi, :],
                                start=(bb == 0),
                                stop=(bb == NB - 1),
                            )
                    # ship any completed range of next layer's table
                    if L < 2:
                        done = (gp + 1) * BPG
                        while shipped < K and (shipped + 1) * R <= done:
                            k = shipped
                            dst_t = g_loc[L + 1][:]
                            dst_ap = bass.AP(
                                dst_t.tensor,
                                k * R * 128 * 64,
                                [[64, 128], [128 * 64, R], [1, 64]],
                            )
                            nc.sync.dma_start(
                                out=dst_ap,
                                in_=G_next[:, k * R : (k + 1) * R, :],
                            )
                            shipped += 1
                if L == 2:
                    pool_sb = sb.tile([NG, D], f32)
                    nc.scalar.activation(
                        out=pool_sb[:], in_=pool_ps[:], func=AF.Copy
                    )
                    nc.sync.dma_start(out=pooled_h[:], in_=pool_sb[:])
    if not nc.is_finalized():
        nc.finalize()
    return nc


LAST_RESULTS = None
LAST_PREP = None


def prepare(inputs):
    x = np.asarray(inputs["x"], np.float32)
    edge_index = np.asarray(inputs["edge_index"])
    batch = np.asarray(inputs["batch"])
    W = [np.asarray(inputs[k], np.float32) for k in ("W1", "W2", "W3")]
    b = [np.asarray(inputs[k], np.float32) for k in ("b1", "b2", "b3")]
    lin_w = np.asarray(inputs["lin_w"], np.float32)
    lin_b = np.asarray(inputs["lin_b"], np.float32)

    dis, bb_of, p_of, T, gidx, dstl = _preprocess(edge_index)

    g0_all = (x @ W[0]) * dis[:, None]  # layer-0 table, exact fp32
    g0 = np.zeros((NC, 128, NB, 64), np.float32)
    g0bb = np.zeros((NC, NB * 128, 64), np.float32)
    disc = np.zeros((NC, 128, NB), np.float32)
    batc = np.full((NC, 128, NB), -1.0, np.float32)
    for c in range(NC):
        nodes = np.arange(c * NPC, (c + 1) * NPC)
        g0[c][p_of[nodes], bb_of[nodes]] = g0_all[nodes]
        g0bb[c][bb_of[nodes] * 128 + p_of[nodes]] = g0_all[nodes]
        disc[c][p_of[nodes], bb_of[nodes]] = dis[nodes]
        batc[c][p_of[nodes], bb_of[nodes]] = batch[nodes].astype(np.float32)

    M = T // 128
    BPGh = max(GI // T, 1)
    import ml_dtypes
    iota = np.ascontiguousarray(
        np.broadcast_to(np.arange(128, dtype=np.float32), (128, BPGh * M, 128))
        .copy()
        .astype(ml_dtypes.bfloat16)
    )
    ident = np.eye(128, dtype=np.float32)
    gid = np.ascontiguousarray(np.tile(np.arange(NG, dtype=np.float32), (128, 1)))
    b_repl = [np.ascontiguousarray(np.tile(bi.reshape(1, D), (128, 1))) for bi in b]
    z = np.zeros((D, D), np.float32)
    wd = [
        np.ascontiguousarray(np.block([[Wi, z], [z, Wi]])) for Wi in (W[1], W[2])
    ]

    nc = _build_program(T)
    in_maps = []
    for c in range(NC):
        in_maps.append(
            {
                "g0": np.ascontiguousarray(g0[c]),
                "g0bb": np.ascontiguousarray(g0bb[c]),
                "disc": np.ascontiguousarray(disc[c]),
                "batc": np.ascontiguousarray(batc[c]),
                "gidx": np.ascontiguousarray(gidx[c]),
                "dstl": np.ascontiguousarray(dstl[c]).astype(ml_dtypes.bfloat16),
                "w0": W[0],
                "w1": W[1],
                "w2": W[2],
                "wd1": wd[0],
                "wd2": wd[1],
                "b0": b_repl[0],
                "b1": b_repl[1],
                "b2": b_repl[2],
                "iota": iota,
                "ident": ident,
                "gid": gid,
            }
        )

    def post(results):
        pooled = np.zeros((NG, D), np.float64)
        for r in results:
            pooled += r["pooled"].astype(np.float64)
        out = pooled.astype(np.float32) @ lin_w + lin_b
        return out.astype(np.float32)

    return nc, in_maps, post


def kernel(**inputs):
    import os
    from concourse.bass_utils import run_bass_kernel_spmd

    nc, in_maps, post = prepare(inputs)
    global LAST_PREP
    LAST_PREP = (nc, in_maps, post)
    trace = os.environ.get("KERNEL_TRACE", "") == "1"
    res = run_bass_kernel_spmd(nc, in_maps, list(range(NC)), trace=trace)
    global LAST_RESULTS
    LAST_RESULTS = res
    return post(res.results)
